# revision 14
# baseline (speedup 1.0000x reference)
"""Trainium2 Bass kernel for nn_InpaintContextAttentionUnit.

Per-sample computation (B=8 samples -> 1 per NeuronCore):
  fm [512,512,16] -> avgpool(64x2) -> pooled [8,256,16]
  -> two masked 3x3 convs (middle row / middle col of kernel zeroed) + bias + relu
  -> bilinear upsample back to [512,512,16] (separable; half-pixel centers, edge clamp)
  -> out [512,512,48] = concat(fm, fm - row_up, fm - col_up)

Design:
  - pooling: PE matmul with a [128,2] block-mean matrix (H-reduce), DVE add for W-pairs
  - conv: per (branch, n-row): 6 accumulating [16c,16f]x[16c,256wp] matmuls in PSUM,
    relu+bias on ACT; taps read from a zero-padded [16, 10n x 258wp] c-major buffer
  - W-upsample (x2, weights .25/.75): folded into 2 strided scalar_tensor_tensor ops
    over an edge-replicated halo buffer
  - H-upsample (x64): PE matmul rw[8n, x] with host-built HUp [8,512] interp matrix
  - combine: DVE subtract (psum - fm strided APs) + ACT copy into interleaved
    [y, x, 48ch] staging tiles, contiguous 3 MiB DMAs out.
All constant matrices are precomputed on host and passed as extra inputs.
"""

import numpy as np

H, W, C, F = 512, 512, 16, 16
NPOOL = 8
WP = W // 2  # 256
CH_OUT = 3 * C  # 48

_cache = {}


def _host_consts(kernel, bias):
    """Build host-side constant matrices."""
    # pooling weights: [128, 2], 1/128 where row block matches
    poolw = np.zeros((128, 2), np.float32)
    poolw[:64, 0] = 1.0 / 128.0
    poolw[64:, 1] = 1.0 / 128.0
    # H-upsample matrix [8, 512]: hup[n, y] = weight of pooled row n for output y
    hup = np.zeros((NPOOL, H), np.float32)
    scale = H // NPOOL
    for y in range(H):
        yf = (y + 0.5) / scale - 0.5
        i0 = int(np.floor(yf))
        w = yf - i0
        hup[min(max(i0, 0), NPOOL - 1), y] += 1.0 - w
        hup[min(max(i0 + 1, 0), NPOOL - 1), y] += w
    # conv taps: branch 0 (row conv): K[dn+1, dwp+1]; branch 1 (col): K[dwp+1, dn+1]
    # tap order: b0: (dn,dwp) for dn in (-1,1) for dwp in (-1,0,1)
    #            b1: (dn,dwp) for dwp in (-1,1) for dn in (-1,0,1)
    taps0 = [(dn, dwp) for dn in (-1, 1) for dwp in (-1, 0, 1)]
    taps1 = [(dn, dwp) for dwp in (-1, 1) for dn in (-1, 0, 1)]
    kt = np.zeros((16, 13 * 16), np.float32)  # [c, (branch*6+tap)*16 + f]; slot 12 = zeros (psum init)
    for i, (dn, dwp) in enumerate(taps0):
        kt[:, i * 16:(i + 1) * 16] = kernel[dn + 1, dwp + 1]
    for i, (dn, dwp) in enumerate(taps1):
        kt[:, (6 + i) * 16:(7 + i) * 16] = kernel[dwp + 1, dn + 1]
    bias2 = np.ascontiguousarray(bias.reshape(16, 1)).astype(np.float32)
    hup2 = np.zeros((40, H), np.float32)
    hup2[0:8] = hup
    hup2[32:40] = hup
    return poolw, hup2, kt, bias2, taps0, taps1


def _build_program(compile=True):
    import concourse.bass as bass
    import concourse.bacc as bacc
    import concourse.mybir as mybir
    import concourse.tile as tile

    dt = mybir.dt.float32
    nc = bacc.Bacc()

    fm_d = nc.declare_dram_parameter("feature_map", [H, W, C], dt, isOutput=False)
    poolw_d = nc.declare_dram_parameter("poolw", [128, 2], dt, isOutput=False)
    hup_d = nc.declare_dram_parameter("hup", [40, H], dt, isOutput=False)
    ktaps_d = nc.declare_dram_parameter("ktaps", [16, 208], dt, isOutput=False)
    bias_d = nc.declare_dram_parameter("bias2", [16, 1], dt, isOutput=False)
    out_d = nc.declare_dram_parameter("out", [H, W, CH_OUT], dt, isOutput=True)

    taps0 = [(dn, dwp) for dn in (-1, 1) for dwp in (-1, 0, 1)]
    taps1 = [(dn, dwp) for dwp in (-1, 1) for dn in (-1, 0, 1)]
    taps_by_branch = [taps0, taps1]

    with tile.TileContext(nc) as tc:
        with (
            tc.tile_pool(name="consts", bufs=1) as cpool,
            tc.tile_pool(name="fm", bufs=2) as fmpool,
            tc.tile_pool(name="persist", bufs=1) as ppool,
        ):
            # ---- load constants ----
            poolw_t = cpool.tile([128, 2], dt)
            nc.sync.dma_start(out=poolw_t[:], in_=poolw_d[:])
            hup_t = cpool.tile([40, H], dt)
            nc.sync.dma_start(out=hup_t[:], in_=hup_d[:])
            ktaps_t = cpool.tile([16, 208], dt)
            nc.sync.dma_start(out=ktaps_t[:], in_=ktaps_d[:])
            bias_t = cpool.tile([16, 1], dt)
            nc.sync.dma_start(out=bias_t[:], in_=bias_d[:])

            # rw [16, 16*512]: partitions 0-7 row-branch n, 8-15 col-branch n;
            # free = (f, x) f-major
            rw_t = ppool.tile([40, 16 * 512], dt)

            # ================= PASS A: pooling + conv + W-upsample =================
            with (
                tc.tile_pool(name="passA", bufs=1) as apool,
                tc.tile_pool(name="dram", bufs=1, space="DRAM") as dpool,
                tc.tile_pool(name="psA", bufs=4, space="PSUM") as psA,
                tc.tile_pool(name="psConv", bufs=2, space="PSUM") as psC,
            ):
                # pooled_T [16 c, (8 n, 258 wp)], c on partitions, zero wp-halo
                # (n-direction zero-padding handled by clipped matmul n-ranges)
                tpad_t = apool.tile([16, NPOOL * 258], dt)
                tpad3 = tpad_t[:].rearrange("p (n w) -> p n w", w=258)

                # pooled_ncw [8 n, (16 c, 256 wp)] c-major
                ncw_t = apool.tile([NPOOL, 16 * WP], dt)
                ncw3 = ncw_t[:].rearrange("p (c w) -> p c w", w=WP)

                for t in range(4):
                    fm_t = fmpool.tile([128, W * C], dt, tag="fm")
                    fm3 = fm_t[:].rearrange("p (x c) -> p x c", c=C)
                    nc.sync.dma_start(out=fm3, in_=fm_d[128 * t:128 * (t + 1)])

                    # stage [2, (c, wp)] c-major on partitions 0-1
                    stage_t = apool.tile([2, 16 * WP], dt, tag="stage")
                    stage3 = stage_t[:].rearrange("p (c w) -> p w c", c=16)  # [2, 256 wp, 16 c]
                    # fm viewed (xp, parity, c): W-pair add folded into PE accumulation
                    fmr = fm_t[:].rearrange("p (xp two c) -> p xp two c", two=2, c=16)
                    for j in range(16):
                        ps = psA.tile([2, 256], dt, tag="pool")
                        for par in range(2):
                            nc.tensor.matmul(
                                ps[:], poolw_t[:],
                                fmr[:, 16 * j:16 * (j + 1), par, :],
                                start=(par == 0), stop=(par == 1),
                            )
                        ps3 = ps[:].rearrange("p (xp c) -> p xp c", c=16)
                        nc.scalar.activation(
                            out=stage3[:, 16 * j:16 * (j + 1), :],
                            in_=ps3,
                            func=mybir.ActivationFunctionType.Copy,
                        )
                    # stage rows (partitions 0-1) -> pooled_ncw rows 2t,2t+1
                    nc.sync.dma_start(out=ncw_t[2 * t:2 * t + 2, :], in_=stage_t[:])

                # pooled_ncw -> pooled_T_pad (c to partitions) via DRAM bounce,
                # adding zero wp-halo columns (zeros sourced from hup rows 8+,
                # which are zero by construction)
                ncw_dram = dpool.tile([NPOOL, 16 * 258], dt)
                nd3 = ncw_dram[:].rearrange("n (c w) -> n c w", w=258)
                ncw3s = ncw_t[:].rearrange("p (c w) -> p c w", w=WP)
                nc.sync.dma_start(out=nd3[:, :, 1:257], in_=ncw3s)
                zsrc = hup_d[8:16, 0:16]  # [8, 16] zeros
                nc.sync.dma_start(out=nd3[:, :, 0:1], in_=zsrc)
                nc.sync.dma_start(out=nd3[:, :, 257:258], in_=zsrc)
                ncwd3 = ncw_dram[:].rearrange("n (c w) -> c n w", w=258)
                nc.sync.dma_start(out=tpad3, in_=ncwd3)

                # ---- conv branches ----
                conv_t = apool.tile([16, 2 * NPOOL * WP], dt, tag="conv_t")  # [16 f, (branch, n, wp)]
                for b in range(2):
                    for ch in range(4):  # n-pair chunks: n in {2ch, 2ch+1}
                        n0 = 2 * ch
                        ps = psC.tile([16, 2 * WP], dt, tag="conv")
                        # zero-init whole chunk (ktaps slot 12 = zeros)
                        nc.tensor.matmul(
                            ps[:], ktaps_t[:, 192:208], tpad3[:, n0:n0 + 2, 1:257],
                            start=True, stop=False, skip_group_check=True,
                        )
                        pieces = []
                        for i, (dn, dwp) in enumerate(taps_by_branch[b]):
                            nlo = max(n0, -dn)
                            nhi = min(n0 + 2, NPOOL - dn)
                            if nhi <= nlo:
                                continue
                            pieces.append((b * 6 + i, dn, dwp, nlo, nhi))
                        for k, (sl, dn, dwp, nlo, nhi) in enumerate(pieces):
                            nc.tensor.matmul(
                                ps[:, (nlo - n0) * WP:(nhi - n0) * WP],
                                ktaps_t[:, sl * 16:(sl + 1) * 16],
                                tpad3[:, nlo + dn:nhi + dn, 1 + dwp:257 + dwp],
                                start=False, stop=(k == len(pieces) - 1),
                                skip_group_check=True,
                            )
                        nc.scalar.activation(
                            out=conv_t[:, (b * NPOOL + n0) * WP:(b * NPOOL + n0 + 2) * WP],
                            in_=ps[:],
                            func=mybir.ActivationFunctionType.Relu,
                            bias=bias_t[:, 0:1],
                        )

                # conv [16 f, (b, n, wp)] -> rop_pad [(b,n) parts, (16 f, 258 wp)]
                # via DRAM bounce (keeps consumer sync fan-in small)
                rop_t = apool.tile([40, 16 * 258], dt)
                rop3 = rop_t[:].rearrange("p (f w) -> p f w", w=258)
                conv_dram = dpool.tile([16, 2 * NPOOL * WP], dt)
                nc.sync.dma_start(out=conv_dram[:], in_=conv_t[:])
                cd4 = conv_dram[:].rearrange("f (b n w) -> b n f w", b=2, n=NPOOL)
                for b in range(2):
                    pg = 32 * b  # partition base: row->0, col->32
                    nc.sync.dma_start(out=rop3[pg:pg + 8, :, 1:257], in_=cd4[b])
                # edge replicate (W clamp)
                for pg in (0, 32):
                    nc.vector.tensor_copy(rop3[pg:pg + 8, :, 0:1], rop3[pg:pg + 8, :, 1:2])
                    nc.vector.tensor_copy(rop3[pg:pg + 8, :, 257:258], rop3[pg:pg + 8, :, 256:257])

                # W-upsample: rw[., f, 2k]   = 0.25*pad[k]   + 0.75*pad[k+1]
                #             rw[., f, 2k+1] = 0.25*pad[k+2] + 0.75*pad[k+1]
                t75_t = apool.tile([40, 16 * 258], dt, tag="conv_t")
                t753 = t75_t[:].rearrange("p (f w) -> p f w", w=258)
                rw4 = rw_t[:].rearrange("p (f x two) -> p f x two", two=2, x=WP)
                for pg in (0, 32):
                    nc.vector.tensor_scalar_mul(
                        t75_t[pg:pg + 8, :], rop_t[pg:pg + 8, :], 0.75)
                    nc.vector.scalar_tensor_tensor(
                        out=rw4[pg:pg + 8, :, :, 0],
                        in0=rop3[pg:pg + 8, :, 0:256],
                        scalar=0.25,
                        in1=t753[pg:pg + 8, :, 1:257],
                        op0=mybir.AluOpType.mult,
                        op1=mybir.AluOpType.add,
                    )
                    nc.vector.scalar_tensor_tensor(
                        out=rw4[pg:pg + 8, :, :, 1],
                        in0=rop3[pg:pg + 8, :, 2:258],
                        scalar=0.25,
                        in1=t753[pg:pg + 8, :, 1:257],
                        op0=mybir.AluOpType.mult,
                        op1=mybir.AluOpType.add,
                    )

            # ================= PASS B: H-upsample + combine + store =================
            with (
                tc.tile_pool(name="passB", bufs=1) as bpool,
                tc.tile_pool(name="psB", bufs=2, space="PSUM") as psB,
            ):
                rwx = rw_t[:].rearrange("p (f x) -> p f x", x=W)
                for t in range(4):
                    fm_t = fmpool.tile([128, W * C], dt, tag="fm")
                    fm3 = fm_t[:].rearrange("p (x c) -> p x c", c=C)
                    nc.sync.dma_start(out=fm3, in_=fm_d[128 * t:128 * (t + 1)])

                    outqs = []
                    for q in range(4):
                        outq_t = bpool.tile([128, 128 * CH_OUT], dt, tag=f"out{q}")
                        outq3 = outq_t[:].rearrange("p (x ch) -> p x ch", ch=CH_OUT)
                        nc.scalar.activation(
                            out=outq3[:, :, 0:16],
                            in_=fm3[:, 128 * q:128 * (q + 1), :],
                            func=mybir.ActivationFunctionType.Copy,
                        )
                        outqs.append((outq_t, outq3))

                    for b in range(2):
                        pg = 32 * b
                        lhsT = hup_t[pg:pg + 8, 128 * t:128 * (t + 1)]  # [8, 128]
                        for fq in range(4):  # f-quads
                            ps = psB.tile([128, 4 * W], dt, tag="up")
                            psf = ps[:].rearrange("p (f x) -> p f x", x=W)
                            for fi in range(4):
                                nc.tensor.matmul(
                                    psf[:, fi, :],
                                    lhsT,
                                    rwx[pg:pg + 8, fq * 4 + fi, :],
                                    start=True, stop=True,
                                )
                            psx = ps[:].rearrange("p (f x) -> p x f", x=W)
                            for q in range(4):
                                nc.vector.tensor_sub(
                                    outqs[q][1][:, :, 16 * (b + 1) + 4 * fq:
                                                16 * (b + 1) + 4 * fq + 4],
                                    fm3[:, 128 * q:128 * (q + 1), 4 * fq:4 * fq + 4],
                                    psx[:, 128 * q:128 * (q + 1), :],
                                )
                    for q in range(4):
                        nc.sync.dma_start(
                            out=out_d[128 * t:128 * (t + 1), 128 * q:128 * (q + 1), :],
                            in_=outqs[q][1],
                        )
    if compile:
        nc.compile()
    return nc


def _get_program():
    if "nc" not in _cache:
        _cache["nc"] = _build_program()
    return _cache["nc"]


def kernel(feature_map, kernel, bias):
    from concourse.bass_utils import run_bass_kernel_spmd

    feature_map = np.ascontiguousarray(feature_map, dtype=np.float32)
    kernel = np.ascontiguousarray(kernel, dtype=np.float32)
    bias = np.ascontiguousarray(bias, dtype=np.float32)
    B = feature_map.shape[0]
    assert B == 8

    poolw, hup, kt, bias2, _, _ = _host_consts(kernel, bias)
    nc = _get_program()
    in_maps = [
        {
            "feature_map": feature_map[b],
            "poolw": poolw,
            "hup": hup,
            "ktaps": kt,
            "bias2": bias2,
        }
        for b in range(B)
    ]
    res = run_bass_kernel_spmd(nc, in_maps, list(range(B)))
    out = np.stack([res.results[b]["out"] for b in range(B)])
    return out


# revision 15
# speedup vs baseline: 1.5195x; 1.5195x over previous
"""Trainium2 Bass kernel for nn_InpaintContextAttentionUnit.

Per-sample computation (B=8 samples -> 1 per NeuronCore):
  fm [512,512,16] -> avgpool(64x2) -> pooled [8,256,16]
  -> two masked 3x3 convs (middle row / middle col of kernel zeroed) + bias + relu
  -> bilinear upsample back to [512,512,16] (separable; half-pixel centers, edge clamp)
  -> out [512,512,48] = concat(fm, fm - row_up, fm - col_up)

Design:
  - pooling: PE matmul with a [128,2] block-mean matrix (H-reduce); W-pair add
    folded into a 2-matmul PSUM accumulation (even/odd x, strided rhs)
  - conv: per (branch, n-pair chunk): zero-init matmul + ~6 accumulating
    [16c,16f]x[16c,<=512] matmuls in PSUM; relu+bias on ACT; taps read from a
    wp-halo'd [16c, 8n x 258wp] buffer assembled via a DRAM bounce
  - W-upsample (x2, weights .25/.75): 2 strided scalar_tensor_tensor ops over an
    edge-replicated halo buffer
  - H-upsample (x64): PE matmul rw[8n, x] with host-built HUp interp matrix
    (row branch at partitions 0-7, col branch at 32-39 per base-partition rules)
  - combine: DVE subtract (fm - psum, strided APs) + ACT copy into interleaved
    [y, x, 48ch] staging tiles, contiguous 3 MiB DMAs out
  - the pooled->conv->upsample chain runs in bf16 (PE bf16 is ~4x faster than the
    fp32-emulation path); PSUM accumulation, fm passthrough, subtract, and the
    output stay fp32
All constant matrices are precomputed on host and passed as extra inputs.
"""

import numpy as np
import ml_dtypes

H, W, C, F = 512, 512, 16, 16
NPOOL = 8
WP = W // 2  # 256
CH_OUT = 3 * C  # 48

_cache = {}


def _host_consts(kernel, bias):
    """Build host-side constant matrices (bf16 for the PE-side constants)."""
    bf = ml_dtypes.bfloat16
    # pooling weights: [128, 2], 1/128 (exact in bf16) where row block matches
    poolw = np.zeros((128, 2), np.float32)
    poolw[:64, 0] = 1.0 / 128.0
    poolw[64:, 1] = 1.0 / 128.0
    # H-upsample matrix: hup[n, y] = weight of pooled row n for output row y
    # (k/64 weights are exact in bf16)
    hup = np.zeros((NPOOL, H), np.float32)
    scale = H // NPOOL
    for y in range(H):
        yf = (y + 0.5) / scale - 0.5
        i0 = int(np.floor(yf))
        w = yf - i0
        hup[min(max(i0, 0), NPOOL - 1), y] += 1.0 - w
        hup[min(max(i0 + 1, 0), NPOOL - 1), y] += w
    hup2 = np.zeros((40, H), np.float32)
    hup2[0:8] = hup
    hup2[32:40] = hup  # col-branch copy at base partition 32
    # conv taps: branch 0 (row conv): K[dn+1, dwp+1]; branch 1 (col): K[dwp+1, dn+1]
    taps0 = [(dn, dwp) for dn in (-1, 1) for dwp in (-1, 0, 1)]
    taps1 = [(dn, dwp) for dwp in (-1, 1) for dn in (-1, 0, 1)]
    kt = np.zeros((16, 13 * 16), np.float32)  # [c, tap*16+f]; slot 12 = zeros
    for i, (dn, dwp) in enumerate(taps0):
        kt[:, i * 16:(i + 1) * 16] = kernel[dn + 1, dwp + 1]
    for i, (dn, dwp) in enumerate(taps1):
        kt[:, (6 + i) * 16:(7 + i) * 16] = kernel[dwp + 1, dn + 1]
    bias2 = np.ascontiguousarray(bias.reshape(16, 1)).astype(np.float32)
    return (poolw.astype(bf), hup2.astype(bf), kt.astype(bf), bias2, taps0, taps1)


def _build_program(compile=True):
    import concourse.bass as bass
    import concourse.bacc as bacc
    import concourse.mybir as mybir
    import concourse.tile as tile

    dt = mybir.dt.float32
    db = mybir.dt.bfloat16
    nc = bacc.Bacc()

    fm_d = nc.declare_dram_parameter("feature_map", [H, W, C], dt, isOutput=False)
    poolw_d = nc.declare_dram_parameter("poolw", [128, 2], db, isOutput=False)
    hup_d = nc.declare_dram_parameter("hup", [40, H], db, isOutput=False)
    ktaps_d = nc.declare_dram_parameter("ktaps", [16, 208], db, isOutput=False)
    bias_d = nc.declare_dram_parameter("bias2", [16, 1], dt, isOutput=False)
    out_d = nc.declare_dram_parameter("out", [H, W, CH_OUT], dt, isOutput=True)

    taps0 = [(dn, dwp) for dn in (-1, 1) for dwp in (-1, 0, 1)]
    taps1 = [(dn, dwp) for dwp in (-1, 1) for dn in (-1, 0, 1)]
    taps_by_branch = [taps0, taps1]

    with tile.TileContext(nc) as tc:
        with (
            tc.tile_pool(name="consts", bufs=1) as cpool,
            tc.tile_pool(name="fm", bufs=2) as fmpool,
            tc.tile_pool(name="persist", bufs=1) as ppool,
        ):
            # ---- load constants ----
            poolw_t = cpool.tile([128, 2], db)
            nc.sync.dma_start(out=poolw_t[:], in_=poolw_d[:])
            hup_t = cpool.tile([40, H], db)
            nc.sync.dma_start(out=hup_t[:], in_=hup_d[:])
            ktaps_t = cpool.tile([16, 208], db)
            nc.sync.dma_start(out=ktaps_t[:], in_=ktaps_d[:])
            bias_t = cpool.tile([16, 1], dt)
            nc.sync.dma_start(out=bias_t[:], in_=bias_d[:])

            # rw [40, (16 f, 512 x)] bf16: partitions 0-7 row-branch, 32-39 col-branch
            rw_t = ppool.tile([40, 16 * 512], db)

            # ================= PASS A: pooling + conv + W-upsample =================
            with (
                tc.tile_pool(name="passA", bufs=1) as apool,
                tc.tile_pool(name="dram", bufs=1, space="DRAM") as dpool,
                tc.tile_pool(name="psA", bufs=4, space="PSUM") as psA,
                tc.tile_pool(name="psConv", bufs=2, space="PSUM") as psC,
            ):
                # pooled_T [16 c, (8 n, 258 wp)] bf16, zero wp-halo; n-direction
                # zero-padding handled by clipped matmul n-ranges
                tpad_t = apool.tile([16, NPOOL * 258], db)
                tpad3 = tpad_t[:].rearrange("p (n w) -> p n w", w=258)

                # pooled_ncw [8 n, (16 c, 256 wp)] bf16, c-major
                ncw_t = apool.tile([NPOOL, 16 * WP], db)

                for t in range(4):
                    # bf16 copy of fm for pooling only (SWDGE cast-DMA)
                    fmb_t = apool.tile([128, W * C], db, tag="fmA")
                    fmb3 = fmb_t[:].rearrange("p (x c) -> p x c", c=C)
                    nc.gpsimd.dma_start(out=fmb3, in_=fm_d[128 * t:128 * (t + 1)])

                    # stage [2, (c, wp)] bf16 on partitions 0-1
                    stage_t = apool.tile([2, 16 * WP], db, tag="stage")
                    stage3 = stage_t[:].rearrange("p (c w) -> p w c", c=16)
                    # fm viewed (xp, parity, c): W-pair add folded into PE accumulation
                    fmr = fmb_t[:].rearrange("p (xp two c) -> p xp two c", two=2, c=16)
                    for j in range(8):  # 32-xp chunks -> N=512
                        ps = psA.tile([2, 512], dt, tag="pool")
                        for par in range(2):
                            nc.tensor.matmul(
                                ps[:], poolw_t[:],
                                fmr[:, 32 * j:32 * (j + 1), par, :],
                                start=(par == 0), stop=(par == 1),
                            )
                        ps3 = ps[:].rearrange("p (xp c) -> p xp c", c=16)
                        nc.scalar.activation(
                            out=stage3[:, 32 * j:32 * (j + 1), :],
                            in_=ps3,
                            func=mybir.ActivationFunctionType.Copy,
                        )
                    nc.sync.dma_start(out=ncw_t[2 * t:2 * t + 2, :], in_=stage_t[:])

                # pooled_ncw -> pooled_T (c to partitions) via DRAM bounce, adding
                # zero wp-halo columns (zeros sourced from hup rows 8-15, zero by
                # construction)
                ncw_dram = dpool.tile([NPOOL, 16 * 258], db)
                nd3 = ncw_dram[:].rearrange("n (c w) -> n c w", w=258)
                ncw3s = ncw_t[:].rearrange("p (c w) -> p c w", w=WP)
                nc.sync.dma_start(out=nd3[:, :, 1:257], in_=ncw3s)
                zsrc = hup_d[8:16, 0:16]  # [8, 16] zeros
                nc.sync.dma_start(out=nd3[:, :, 0:1], in_=zsrc)
                nc.sync.dma_start(out=nd3[:, :, 257:258], in_=zsrc)
                ncwd3 = ncw_dram[:].rearrange("n (c w) -> c n w", w=258)
                nc.sync.dma_start(out=tpad3, in_=ncwd3)

                # ---- conv branches ----
                conv_t = apool.tile([16, 2 * NPOOL * WP], db, tag="conv_t")
                for b in range(2):
                    for ch in range(4):  # n-pair chunks: n in {2ch, 2ch+1}
                        n0 = 2 * ch
                        ps = psC.tile([16, 2 * WP], dt, tag="conv")
                        # zero-init whole chunk (ktaps slot 12 = zeros)
                        nc.tensor.matmul(
                            ps[:], ktaps_t[:, 192:208], tpad3[:, n0:n0 + 2, 1:257],
                            start=True, stop=False, skip_group_check=True,
                        )
                        pieces = []
                        for i, (dn, dwp) in enumerate(taps_by_branch[b]):
                            nlo = max(n0, -dn)
                            nhi = min(n0 + 2, NPOOL - dn)
                            if nhi <= nlo:
                                continue
                            pieces.append((b * 6 + i, dn, dwp, nlo, nhi))
                        for k, (sl, dn, dwp, nlo, nhi) in enumerate(pieces):
                            nc.tensor.matmul(
                                ps[:, (nlo - n0) * WP:(nhi - n0) * WP],
                                ktaps_t[:, sl * 16:(sl + 1) * 16],
                                tpad3[:, nlo + dn:nhi + dn, 1 + dwp:257 + dwp],
                                start=False, stop=(k == len(pieces) - 1),
                                skip_group_check=True,
                            )
                        nc.scalar.activation(
                            out=conv_t[:, (b * NPOOL + n0) * WP:(b * NPOOL + n0 + 2) * WP],
                            in_=ps[:],
                            func=mybir.ActivationFunctionType.Relu,
                            bias=bias_t[:, 0:1],
                        )

                # conv [16 f, (b, n, wp)] -> rop_pad [(b,n) parts, (16 f, 258 wp)]
                # via DRAM bounce (keeps consumer sync fan-in small)
                rop_t = apool.tile([40, 16 * 258], db)
                rop3 = rop_t[:].rearrange("p (f w) -> p f w", w=258)
                conv_dram = dpool.tile([16, 2 * NPOOL * WP], db)
                nc.sync.dma_start(out=conv_dram[:], in_=conv_t[:])
                cd4 = conv_dram[:].rearrange("f (b n w) -> b n f w", b=2, n=NPOOL)
                for b in range(2):
                    pg = 32 * b  # partition base: row->0, col->32
                    nc.sync.dma_start(out=rop3[pg:pg + 8, :, 1:257], in_=cd4[b])
                # edge replicate (W clamp)
                for pg in (0, 32):
                    nc.vector.tensor_copy(rop3[pg:pg + 8, :, 0:1], rop3[pg:pg + 8, :, 1:2])
                    nc.vector.tensor_copy(rop3[pg:pg + 8, :, 257:258], rop3[pg:pg + 8, :, 256:257])

                # W-upsample: rw[., f, 2k]   = 0.25*pad[k]   + 0.75*pad[k+1]
                #             rw[., f, 2k+1] = 0.25*pad[k+2] + 0.75*pad[k+1]
                t75_t = apool.tile([40, 16 * 258], db, tag="conv_t")
                t753 = t75_t[:].rearrange("p (f w) -> p f w", w=258)
                rw4 = rw_t[:].rearrange("p (f x two) -> p f x two", two=2, x=WP)
                for pg in (0, 32):
                    nc.vector.tensor_scalar_mul(
                        t75_t[pg:pg + 8, :], rop_t[pg:pg + 8, :], 0.75)
                    nc.vector.scalar_tensor_tensor(
                        out=rw4[pg:pg + 8, :, :, 0],
                        in0=rop3[pg:pg + 8, :, 0:256],
                        scalar=0.25,
                        in1=t753[pg:pg + 8, :, 1:257],
                        op0=mybir.AluOpType.mult,
                        op1=mybir.AluOpType.add,
                    )
                    nc.vector.scalar_tensor_tensor(
                        out=rw4[pg:pg + 8, :, :, 1],
                        in0=rop3[pg:pg + 8, :, 2:258],
                        scalar=0.25,
                        in1=t753[pg:pg + 8, :, 1:257],
                        op0=mybir.AluOpType.mult,
                        op1=mybir.AluOpType.add,
                    )

            # ================= PASS B: H-upsample + combine + store =================
            with (
                tc.tile_pool(name="passB", bufs=1) as bpool,
                tc.tile_pool(name="psB", bufs=2, space="PSUM") as psB,
            ):
                rwx = rw_t[:].rearrange("p (f x) -> p f x", x=W)
                for t in range(4):
                    fm_t = fmpool.tile([128, W * C], dt, tag="fm")
                    fm3 = fm_t[:].rearrange("p (x c) -> p x c", c=C)
                    nc.sync.dma_start(out=fm3, in_=fm_d[128 * t:128 * (t + 1)])

                    outqs = []
                    for q in range(4):
                        outq_t = bpool.tile([128, 128 * CH_OUT], dt, tag=f"out{q}")
                        outq3 = outq_t[:].rearrange("p (x ch) -> p x ch", ch=CH_OUT)
                        nc.scalar.activation(
                            out=outq3[:, :, 0:16],
                            in_=fm3[:, 128 * q:128 * (q + 1), :],
                            func=mybir.ActivationFunctionType.Copy,
                        )
                        outqs.append(outq3)

                    for b in range(2):
                        pg = 32 * b
                        lhsT = hup_t[pg:pg + 8, 128 * t:128 * (t + 1)]  # [8, 128]
                        for fq in range(4):  # f-quads
                            ps = psB.tile([128, 4 * W], dt, tag="up")
                            psf = ps[:].rearrange("p (f x) -> p f x", x=W)
                            for fi in range(4):
                                nc.tensor.matmul(
                                    psf[:, fi, :],
                                    lhsT,
                                    rwx[pg:pg + 8, fq * 4 + fi, :],
                                    start=True, stop=True,
                                )
                            psx = ps[:].rearrange("p (f x) -> p x f", x=W)
                            for q in range(4):
                                nc.vector.tensor_sub(
                                    outqs[q][:, :, 16 * (b + 1) + 4 * fq:
                                             16 * (b + 1) + 4 * fq + 4],
                                    fm3[:, 128 * q:128 * (q + 1), 4 * fq:4 * fq + 4],
                                    psx[:, 128 * q:128 * (q + 1), :],
                                )
                    for q in range(4):
                        nc.sync.dma_start(
                            out=out_d[128 * t:128 * (t + 1), 128 * q:128 * (q + 1), :],
                            in_=outqs[q],
                        )
    if compile:
        nc.compile()
    return nc


def _get_program():
    if "nc" not in _cache:
        _cache["nc"] = _build_program()
    return _cache["nc"]


def kernel(feature_map, kernel, bias):
    from concourse.bass_utils import run_bass_kernel_spmd

    feature_map = np.ascontiguousarray(feature_map, dtype=np.float32)
    kernel = np.ascontiguousarray(kernel, dtype=np.float32)
    bias = np.ascontiguousarray(bias, dtype=np.float32)
    B = feature_map.shape[0]
    assert B == 8

    poolw, hup, kt, bias2, _, _ = _host_consts(kernel, bias)
    nc = _get_program()
    in_maps = [
        {
            "feature_map": feature_map[b],
            "poolw": poolw,
            "hup": hup,
            "ktaps": kt,
            "bias2": bias2,
        }
        for b in range(B)
    ]
    res = run_bass_kernel_spmd(nc, in_maps, list(range(B)))
    out = np.stack([res.results[b]["out"] for b in range(B)])
    return out
